# revision 4
# baseline (speedup 1.0000x reference)
"""NeighborhoodAttention1D (NATTEN, kernel=13) fused Trainium2 kernel.

Strategy: sequence-parallel over 8 NeuronCores. Each core handles a 512-token
shard (both batches) plus a 6-token halo on each side, so no collectives are
needed: QKV projection, banded attention, and output projection all run
locally; the host slices inputs (with halo) and concatenates outputs.

Per core (all matmuls bf16, fp32 PSUM accumulation):
  phase 1a: Q^T tiles  [128c x (B*512)]  = w_q'^T x   (scale folded into w_q)
  phase 1b: K^T tiles  [128c x (B*524)]  = w_k^T x    (halo tokens)
  phase 1c: V natural  [128tok x 1024]   = x^T w_v    (halo tokens)
  phase 2:  per (batch, 128-query block, head): dense 128x140 score block
            against the key window, additive mask+rpb (host precomputed),
            softmax (max-subtract, exp with accumulated row sum), PE-transpose
            of probabilities, PV matmuls -> O^T tiles [128c x (B*512)]
  phase 3:  y^T = w_proj^T O  -> DMA out as [1024, B*512] fp32

The NATTEN window clamp (queries near sequence edges) is folded into the
host-precomputed additive mask tensor, which is per-core data, so one NEFF
serves all 8 cores.
"""

import sys

if '/opt/trn_rl_repo' not in sys.path:
    sys.path.insert(0, '/opt/trn_rl_repo')

import numpy as np
import ml_dtypes
from contextlib import ExitStack

import concourse.bacc as bacc
import concourse.mybir as mybir
import concourse.tile as tile
from concourse import bass_utils
from concourse.masks import make_identity

F32 = mybir.dt.float32
BF16 = mybir.dt.bfloat16
AF = mybir.ActivationFunctionType

B, L, C, H, D, KK = 2, 4096, 1024, 16, 64, 13
NCORES = 8
LS = L // NCORES          # 512 queries per core per batch
QOFF = (KK - 1) // 2      # 6
LH = LS + 2 * QOFF        # 524 halo tokens per core per batch
NBLK = LS // 128          # 4 query blocks
WIN = 128 + KK - 1        # 140 key window per block
KT = C // 128             # 8 contraction tiles
SCALE = D ** -0.5

_CACHE = {}


def _build_program():
    nc = bacc.Bacc("TRN2", target_bir_lowering=False, debug=False,
                   num_devices=NCORES)

    xt_d = nc.dram_tensor("xt", [128, KT, B * LH], BF16, kind="ExternalInput").ap()
    wqk_d = nc.dram_tensor("wqk", [128, 16, KT, 128], BF16, kind="ExternalInput").ap()
    wv_d = nc.dram_tensor("wv", [128, KT, C], BF16, kind="ExternalInput").ap()
    wp_d = nc.dram_tensor("wp", [128, 8, KT, 128], BF16, kind="ExternalInput").ap()
    am_d = nc.dram_tensor("am", [128, H * NBLK * WIN], BF16, kind="ExternalInput").ap()
    bqk_d = nc.dram_tensor("bqk", [128, 16], F32, kind="ExternalInput").ap()
    bv_d = nc.dram_tensor("bv", [1, C], BF16, kind="ExternalInput").ap()
    bp_d = nc.dram_tensor("bp", [128, 8], F32, kind="ExternalInput").ap()
    yt_d = nc.dram_tensor("yt", [C, B * LS], F32, kind="ExternalOutput").ap()

    with tile.TileContext(nc) as tc, ExitStack() as ctx:
        pers = ctx.enter_context(tc.tile_pool(name="pers", bufs=1))

        ident = pers.tile([128, 128], BF16, tag="ident")
        make_identity(nc, ident[:])
        ones = pers.tile([1, 128], BF16, tag="ones")
        nc.vector.memset(ones[:], 1.0)

        # Big loads are split into ~0.5 MB pieces: walrus expands each DMA
        # into ~1 KB descriptor units that tick the queue semaphore +16, and
        # a single multi-MB DMA overflows the 16-bit sem-wait ISA field.
        xt = pers.tile([128, KT, B * LH], BF16, tag="xt")
        for k in range(KT):
            nc.sync.dma_start(xt[:, k], xt_d[:, k])
        wqk = pers.tile([128, 16, KT, 128], BF16, tag="wqk")
        for m in range(0, 16, 2):
            nc.sync.dma_start(wqk[:, m:m + 2], wqk_d[:, m:m + 2])
        wv = pers.tile([128, KT, C], BF16, tag="wv")
        for k in range(0, KT, 2):
            nc.sync.dma_start(wv[:, k:k + 2], wv_d[:, k:k + 2])
        wp = pers.tile([128, 8, KT, 128], BF16, tag="wp")
        for m in range(0, 8, 2):
            nc.sync.dma_start(wp[:, m:m + 2], wp_d[:, m:m + 2])
        am = pers.tile([128, H * NBLK * WIN], BF16, tag="am")
        ACH = H * NBLK * WIN // 4
        for q in range(4):
            nc.sync.dma_start(am[:, q * ACH:(q + 1) * ACH],
                              am_d[:, q * ACH:(q + 1) * ACH])
        bqk = pers.tile([128, 16], F32, tag="bqk")
        nc.sync.dma_start(bqk[:], bqk_d)
        bv = pers.tile([1, C], BF16, tag="bv")
        nc.sync.dma_start(bv[:], bv_d)
        bp = pers.tile([128, 8], F32, tag="bp")
        nc.sync.dma_start(bp[:], bp_d)

        qt = [pers.tile([128, B * LS], BF16, name=f"qt{m}", tag=f"qt{m}") for m in range(8)]
        kt = [pers.tile([128, B * LH], BF16, name=f"kt{m}", tag=f"kt{m}") for m in range(8)]
        vt = [[pers.tile([128, C], BF16, name=f"vt{b}_{t}", tag=f"vt{b}_{t}") for t in range(NBLK + 1)]
              for b in range(B)]
        ot = [pers.tile([128, B * LS], BF16, name=f"ot{m}", tag=f"ot{m}") for m in range(8)]

        # ---- phase 1a: Q^T (scale pre-folded into weights/bias on host) ----
        with tc.tile_pool(name="psq", bufs=2, space="PSUM") as psq:
            for mt in range(8):
                for b in range(B):
                    ps = psq.tile([128, LS], F32)
                    for k in range(KT):
                        nc.tensor.matmul(
                            ps[:], wqk[:, mt, k, :],
                            xt[:, k, b * LH + QOFF: b * LH + QOFF + LS],
                            start=(k == 0), stop=(k == KT - 1))
                    nc.scalar.activation(qt[mt][:, b * LS:(b + 1) * LS], ps[:],
                                         AF.Identity, bias=bqk[:, mt:mt + 1])

        # ---- phase 1b: K^T over full halo ----
        with tc.tile_pool(name="psk", bufs=2, space="PSUM") as psk:
            for mt in range(8):
                for b in range(B):
                    ps = psk.tile([128, LH], F32)
                    for k in range(KT):
                        nc.tensor.matmul(
                            ps[:, 0:512], wqk[:, 8 + mt, k, :],
                            xt[:, k, b * LH: b * LH + 512],
                            start=(k == 0), stop=(k == KT - 1))
                    for k in range(KT):
                        nc.tensor.matmul(
                            ps[:, 512:LH], wqk[:, 8 + mt, k, :],
                            xt[:, k, b * LH + 512: b * LH + LH],
                            start=(k == 0), stop=(k == KT - 1))
                    nc.scalar.activation(kt[mt][:, b * LH:(b + 1) * LH], ps[:],
                                         AF.Identity, bias=bqk[:, 8 + mt:9 + mt])

        # ---- phase 1c: V natural layout [tokens, C] ----
        with tc.tile_pool(name="psv", bufs=2, space="PSUM") as psv:
            for b in range(B):
                for tb in range(NBLK + 1):
                    rows = 128 if tb < NBLK else LH - NBLK * 128
                    for half in range(2):
                        ps = psv.tile([128, 512], F32)
                        nc.tensor.matmul(ps[0:rows, :], ones[0:1, 0:rows],
                                         bv[0:1, half * 512:(half + 1) * 512],
                                         start=True, stop=False)
                        for k in range(KT):
                            nc.tensor.matmul(
                                ps[0:rows, :],
                                xt[:, k, b * LH + tb * 128: b * LH + tb * 128 + rows],
                                wv[:, k, half * 512:(half + 1) * 512],
                                start=False, stop=(k == KT - 1))
                        nc.vector.tensor_copy(
                            vt[b][tb][0:rows, half * 512:(half + 1) * 512],
                            ps[0:rows, :])

        # ---- phase 2: banded attention ----
        with tc.tile_pool(name="attn", bufs=3) as apool, \
             tc.tile_pool(name="psa_s", bufs=2, space="PSUM") as psa_s, \
             tc.tile_pool(name="psa_t", bufs=2, space="PSUM") as psa_t, \
             tc.tile_pool(name="psa_o", bufs=2, space="PSUM") as psa_o:
            for b in range(B):
                for blk in range(NBLK):
                    for h in range(H):
                        mt, po = h // 2, (h % 2) * 64
                        s_ps = psa_s.tile([128, WIN], F32)
                        nc.tensor.matmul(
                            s_ps[:],
                            qt[mt][po:po + 64, b * LS + blk * 128: b * LS + blk * 128 + 128],
                            kt[mt][po:po + 64, b * LH + blk * 128: b * LH + blk * 128 + WIN],
                            start=True, stop=True)
                        s_sb = apool.tile([128, WIN], BF16, tag="s_sb")
                        nc.vector.tensor_add(
                            s_sb[:], s_ps[:],
                            am[:, (h * NBLK + blk) * WIN: (h * NBLK + blk + 1) * WIN])
                        negmax = apool.tile([128, 1], F32, tag="negmax")
                        nc.vector.tensor_reduce(negmax[:], s_sb[:],
                                                axis=mybir.AxisListType.X,
                                                op=mybir.AluOpType.max, negate=True)
                        p_sb = apool.tile([128, WIN], BF16, tag="p_sb")
                        rowsum = apool.tile([128, 1], F32, tag="rowsum")
                        nc.scalar.activation(p_sb[:], s_sb[:], AF.Exp,
                                             bias=negmax[:], scale=1.0,
                                             accum_out=rowsum[:])
                        rsum = apool.tile([128, 1], F32, tag="rsum")
                        nc.vector.reciprocal(rsum[:], rowsum[:])
                        pn = apool.tile([128, WIN], BF16, tag="pn")
                        nc.vector.tensor_scalar_mul(pn[:], p_sb[:], rsum[:])
                        pt = psa_t.tile([128, 256], BF16)
                        nc.tensor.transpose(pt[:, 0:128], pn[:, 0:128], ident[:])
                        nc.tensor.transpose(pt[0:KK - 1, 128:256], pn[:, 128:WIN],
                                            ident[:])
                        st = apool.tile([128, 256], BF16, tag="st")
                        nc.vector.tensor_copy(st[:], pt[:])
                        o_ps = psa_o.tile([64, 128], F32)
                        nc.tensor.matmul(o_ps[:],
                                         vt[b][blk][:, h * 64:(h + 1) * 64],
                                         st[:, 0:128], start=True, stop=False)
                        nc.tensor.matmul(o_ps[:],
                                         vt[b][blk + 1][0:KK - 1, h * 64:(h + 1) * 64],
                                         st[0:KK - 1, 128:256],
                                         start=False, stop=True)
                        nc.scalar.copy(
                            ot[mt][po:po + 64, b * LS + blk * 128: b * LS + blk * 128 + 128],
                            o_ps[:])

        # ---- phase 3: output projection, y^T ----
        with tc.tile_pool(name="psp", bufs=2, space="PSUM") as psp, \
             tc.tile_pool(name="yout", bufs=2) as yout:
            for mt in range(8):
                for ch in range(2):
                    ps = psp.tile([128, 512], F32)
                    for k in range(KT):
                        nc.tensor.matmul(ps[:], wp[:, mt, k, :],
                                         ot[k][:, ch * 512:(ch + 1) * 512],
                                         start=(k == 0), stop=(k == KT - 1))
                    y_sb = yout.tile([128, 512], F32)
                    nc.scalar.activation(y_sb[:], ps[:], AF.Identity,
                                         bias=bp[:, mt:mt + 1])
                    nc.sync.dma_start(
                        yt_d[mt * 128:(mt + 1) * 128, ch * 512:(ch + 1) * 512],
                        y_sb[:])

    nc.compile()
    return nc


def _amask_for_core(rpb, s):
    """Additive mask+rpb tensor [128, H*NBLK*WIN] bf16 for shard s."""
    l = np.arange(LS)
    pos = s * LS + l
    start_g = np.clip(pos - QOFF, 0, L - KK)
    base = s * LS - QOFF
    cs = start_g - base - (l // 128) * 128
    j = np.arange(KK)
    cols = cs[:, None] + j[None, :]                       # [LS, KK]
    bidx = start_g[:, None] + j[None, :] - pos[:, None] + (KK - 1)
    A = np.full((H, LS, WIN), -1e4, np.float32)
    li = np.repeat(l, KK)
    A[:, li, cols.ravel()] = rpb[:, bidx.ravel()]
    A4 = A.reshape(H, NBLK, 128, WIN)
    return np.ascontiguousarray(
        A4.transpose(2, 0, 1, 3).reshape(128, H * NBLK * WIN)
    ).astype(ml_dtypes.bfloat16)


def kernel(x, w_qkv, b_qkv, rpb, w_proj, b_proj, k_neighbors):
    x = np.asarray(x, np.float32)
    w_qkv = np.asarray(w_qkv, np.float32)
    b_qkv = np.asarray(b_qkv, np.float32)
    rpb = np.asarray(rpb, np.float32)
    w_proj = np.asarray(w_proj, np.float32)
    b_proj = np.asarray(b_proj, np.float32)
    assert int(k_neighbors) == 144 and x.shape == (B, L, C)

    if "nc" not in _CACHE:
        _CACHE["nc"] = _build_program()
    nc = _CACHE["nc"]

    bf = ml_dtypes.bfloat16
    w_q = w_qkv[:, :C] * SCALE
    w_k = w_qkv[:, C:2 * C]
    w_v = w_qkv[:, 2 * C:]
    wqk_cat = np.concatenate([w_q, w_k], axis=1)          # [C, 2C]
    wqk_p = np.ascontiguousarray(
        wqk_cat.reshape(KT, 128, 16, 128).transpose(1, 2, 0, 3)).astype(bf)
    wv_p = np.ascontiguousarray(
        w_v.reshape(KT, 128, C).transpose(1, 0, 2)).astype(bf)
    wp_p = np.ascontiguousarray(
        w_proj.reshape(KT, 128, 8, 128).transpose(1, 2, 0, 3)).astype(bf)
    bqk_cat = np.concatenate([b_qkv[:C] * SCALE, b_qkv[C:2 * C]])
    bqk_p = np.ascontiguousarray(bqk_cat.reshape(16, 128).T).astype(np.float32)
    bv_p = b_qkv[2 * C:].reshape(1, C).astype(bf)
    bp_p = np.ascontiguousarray(b_proj.reshape(8, 128).T).astype(np.float32)

    x_pad = np.zeros((B, L + 2 * QOFF, C), np.float32)
    x_pad[:, QOFF:QOFF + L] = x

    in_maps = []
    for s in range(NCORES):
        xh = x_pad[:, s * LS: s * LS + LH]                 # [B, LH, C]
        xt_p = np.ascontiguousarray(
            xh.transpose(2, 0, 1).reshape(KT, 128, B * LH).transpose(1, 0, 2)
        ).astype(bf)
        in_maps.append({
            "xt": xt_p, "wqk": wqk_p, "wv": wv_p, "wp": wp_p,
            "am": _amask_for_core(rpb, s),
            "bqk": bqk_p, "bv": bv_p, "bp": bp_p,
        })

    res = bass_utils.run_bass_kernel_spmd(nc, in_maps,
                                          core_ids=list(range(NCORES)),
                                          **_CACHE.get("run_kwargs", {}))
    _CACHE["last_res"] = res
    y = np.empty((B, L, C), np.float32)
    for s in range(NCORES):
        yts = res.results[s]["yt"]                         # [C, B*LS]
        y[:, s * LS:(s + 1) * LS] = yts.reshape(C, B, LS).transpose(1, 2, 0)
    return y
